# revision 20
# baseline (speedup 1.0000x reference)
"""BertSelfAttention on 8 Trainium2 NeuronCores (~330us HW).

Sharding: 8 cores = 4 batches x 2 head-halves. Each core computes, for its
batch b and its 8 heads, the unnormalized attention output transposed
(out.T = V.T @ P.T per head) plus the softmax denominator row (via a ones
column appended to V). The host pre-transposes inputs (X.T, W.T slices,
cast to fp16) and does the final normalize/transpose/concat.

Single software-pipelined stream per core:
- Q/K projections for head-pair chunk 0 run first; V-projection tiles are
  interleaved one-per-k-step into pair 0's first k-loop just ahead of the
  AV consumer; chunk c+1's projections run as one contiguous seam between
  qb0 and qb1 of pair c (their PSUM casts deferred one tile so they never
  idle at an engine FIFO head).
- Score matmuls (K=64, fp16) pair heads p0/p1 on PE row groups 0-63 and
  64-127 (stationary and moving at matching partition offsets) so both
  stream concurrently. Each (p, q2) score half gets its own 1-bank PSUM
  tile, keeping the scores->exp->scores WAR chain short.
- The exp is split across engines: p0 takes exact exp on ScalarE
  (ACTIVATE, scale=1/8 fused); p1 takes a Schraudolph-style bit-trick exp
  on VectorE (one fused tensor_scalar: round(x*C1+C2) -> int16, bitcast
  fp16), with the shift constant tuned to zero the mean multiplicative
  error so softmax normalization cancels the bias (rel err ~9e-3).
- AV matmuls lag scores by 4 k-steps and the pending queue carries across
  q-block boundaries, filling the scores-only (exp-chain-bound) k-steps at
  each block start; output evacuations burst after the carried tail AVs
  and before the next block's av(0) reuses the po accumulators.
"""

import sys

if "/opt/trn_rl_repo" not in sys.path:
    sys.path.insert(0, "/opt/trn_rl_repo")

import numpy as np

import concourse.bass as bass  # noqa: F401  (registers bass machinery)
import concourse.tile as tile
from concourse import bacc, mybir
from concourse.bass_utils import run_bass_kernel_spmd

B, S, H = 4, 2048, 1024
NH, DH = 16, 64
NCORES = 8
HPC = 8            # heads per core
OC = HPC * DH      # 512 output features per core
HC = H // 128      # 8 contraction chunks of 128
DHE = DH + 1       # head dim + denominator column

F16 = mybir.dt.float16
F32 = mybir.dt.float32
I16 = mybir.dt.int16
EXP = mybir.ActivationFunctionType.Exp
MULT = mybir.AluOpType.mult
ADD = mybir.AluOpType.add

# bit-trick exp constants: round(s*0.125*log2(e)*1024 + (15+delta)*1024),
# bitcast int16 -> fp16 ~= exp(s/8) * (1 + eps), E[eps] = 0 via delta.
TRICK_DELTA = -0.05753274588835121
TRICK_C1 = 0.125 * np.log2(np.e) * 1024.0
TRICK_C2 = (15.0 + TRICK_DELTA) * 1024.0


_PROGRAM = None
LAST_RESULT = None  # BassKernelResults of the most recent kernel() call


def _emit_kernel(tc, out, xt, wqt, wkt, wvt):
    nc = tc.nc
    with (
        tc.tile_pool(name="persist", bufs=1) as persist,
        tc.tile_pool(name="ptp", bufs=8) as ptp,
        tc.tile_pool(name="ost", bufs=4) as ost,
        # one PSUM pool, 4 tags: ps0/ps1 double-buffer scores (and host the
        # interleaved projection tiles); po0/po1 are the per-head AV
        # accumulators, live across each k-loop.
        tc.tile_pool(name="psa", bufs=1, space="PSUM") as psa,
    ):
        xt_sb = persist.tile([128, HC, S], F16)
        wq_sb = persist.tile([128, HC, OC], F16)
        wk_sb = persist.tile([128, HC, OC], F16)
        wv_sb = persist.tile([128, HC, OC], F16)
        qt_sb = persist.tile([128, 4, S], F16)
        kt_sb = persist.tile([128, 4, S], F16)
        v_sb = persist.tile([128, 16, HPC * DHE], F16)

        xt_chunks = xt.rearrange("(c p) s -> p c s", p=128)
        wq_chunks = wqt.rearrange("(c p) o -> p c o", p=128)
        wk_chunks = wkt.rearrange("(c p) o -> p c o", p=128)
        wv_chunks = wvt.rearrange("(c p) o -> p c o", p=128)

        # chunk-0 W slices first (QK proj chunk 0 starts the pipeline),
        # then X, then the rest.
        nc.sync.dma_start(wq_sb[:, :, 0:128], wq_chunks[:, :, 0:128])
        nc.sync.dma_start(wk_sb[:, :, 0:128], wk_chunks[:, :, 0:128])
        for hc in range(HC):
            nc.sync.dma_start(xt_sb[:, hc, :], xt_chunks[:, hc, :])
        for hc in range(HC):
            nc.sync.dma_start(wv_sb[:, hc, :], wv_chunks[:, hc, :])
        nc.sync.dma_start(wq_sb[:, :, 128:OC], wq_chunks[:, :, 128:OC])
        nc.sync.dma_start(wk_sb[:, :, 128:OC], wk_chunks[:, :, 128:OC])

        # fill V with ones first; projection copies overwrite the data columns,
        # leaving a ones column per head to accumulate softmax denominators
        nc.vector.memset(v_sb[:], 1.0)

        # score PSUM tiles: 4 tags (p, q2) of [128, 512], one bank each; the
        # interleaved projection tiles rotate over the same tags.
        PSTAGS = ("ps00", "ps01", "ps10", "ps11")

        def proj_tile(idx, w_sb, c, sc, dst, defer=None):
            # emits the 8 accumulating matmuls; the PSUM->SBUF cast emission
            # is appended to `defer` so the caller can place it after the
            # k-step's exp ops (a cast emitted earlier would head-of-line
            # block its engine's FIFO queue while waiting on the matmuls).
            tag = PSTAGS[idx % 4]
            p = psa.tile([128, 512], F32, tag=tag, name=f"pp_{tag}")
            for hc in range(HC):
                nc.tensor.matmul(
                    p[:],
                    w_sb[:, hc, c * 128 : (c + 1) * 128],
                    xt_sb[:, hc, sc * 512 : (sc + 1) * 512],
                    start=(hc == 0),
                    stop=(hc == HC - 1),
                )

            def cast():
                nc.scalar.copy(dst[:, c, sc * 512 : (sc + 1) * 512], p[:])

            if defer is None:
                cast()
            else:
                defer.append(cast)

        def v_tile(idx, st, defer=None):
            p = psa.tile([128, 512], F32, tag=PSTAGS[idx % 4], name=f"pv{idx % 4}")
            for hc in range(HC):
                nc.tensor.matmul(
                    p[:],
                    xt_sb[:, hc, st * 128 : (st + 1) * 128],
                    wv_sb[:, hc, :],
                    start=(hc == 0),
                    stop=(hc == HC - 1),
                )

            def cast():
                nc.vector.tensor_copy(
                    v_sb[:, st, :].rearrange("p (h e) -> p h e", e=DHE)[:, :, 0:DH],
                    p[:].rearrange("p (h d) -> p h d", d=DH),
                )

            if defer is None:
                cast()
            else:
                defer.append(cast)

        def qk_tile(n, c, sc_idx, defer=None):
            # sc_idx 0..7: q projections for sc 0..3, then k projections
            w_sb, dst = (wq_sb, qt_sb) if sc_idx < 4 else (wk_sb, kt_sb)
            proj_tile(n, w_sb, c, sc_idx % 4, dst, defer=defer)

        # ---- prologue: Q/K projections for chunk 0, then V tile 0 ----
        n = 0
        for sc_idx in range(8):
            qk_tile(n, 0, sc_idx)
            n += 1
        v_tile(n, 0)
        n += 1

        # ---- pipelined attention: pair c, with chunk c+1 proj (and, for
        # pair 0, the V tiles) interleaved into the k-loops ----
        def av_mms(pair, po, k, pts, q2):
            for p in range(2):
                hsl = slice((2 * pair + p) * DHE, (2 * pair + p + 1) * DHE)
                nc.tensor.matmul(
                    po[p][:, q2 * 512 : (q2 + 1) * 512],
                    v_sb[:, k, hsl],
                    pts[p][:, q2 * 512 : (q2 + 1) * 512],
                    start=(k == 0),
                    stop=(k == 15),
                )

        pending_out = []   # previous (pair, qb) output evacuation closures
        pending = []       # AV closures, carried ACROSS qb boundaries so the
                           # previous block's tail AVs fill the next block's
                           # first (scores-only, exp-chain-bound) k-steps
        for pair in range(HPC // 2):
            chunk = pair
            for qb in range(2):         # q blocks of 1024
                po = [psa.tile([DHE, 1024], F32, tag=f"po{p}", name=f"po{p}") for p in range(2)]
                for k in range(16):     # key tiles of 128
                    ksl = slice(k * 128, (k + 1) * 128)
                    pt = [ptp.tile([128, 1024], F16, tag=f"pt{p}", name=f"pt{p}") for p in range(2)]
                    # AV lags scores by 4 k-steps (pt buffering decouples the
                    # AV stream from exp-engine transients); the carried AV's
                    # two q2 halves interleave between this k-step's score
                    # pairs, giving the score LDWEIGHTS a window to
                    # background-load while the AV halves stream.
                    avfn = pending.pop(0) if len(pending) > 3 else None
                    # the p0/p1 score matmuls use disjoint PE row groups
                    # (rows 0-63 / 64-127 for stationary and moving) and run
                    # concurrently. Each (p, q2) half gets its own 1-bank
                    # PSUM tile; p0's exp is exact on ScalarE, p1's is the
                    # bit-trick on VectorE (deterministic queues).
                    for q2 in range(2):
                        q0 = qb * 1024 + q2 * 512
                        ps = [
                            psa.tile([128, 512], F32, tag=f"ps{p}{q2}", name=f"ps{p}{q2}")
                            for p in range(2)
                        ]
                        for p in range(2):  # head parity: rows 0-63 / 64-127
                            base = p * 64
                            nc.tensor.matmul(
                                ps[p][:],
                                kt_sb[base : base + 64, chunk, ksl],
                                qt_sb[base : base + 64, chunk, q0 : q0 + 512],
                                start=True,
                                stop=True,
                            )
                        nc.scalar.activation(
                            pt[0][:, q2 * 512 : (q2 + 1) * 512], ps[0][:],
                            EXP, scale=0.125,
                        )
                        nc.vector.tensor_scalar(
                            pt[1][:, q2 * 512 : (q2 + 1) * 512].bitcast(I16),
                            ps[1][:], TRICK_C1, TRICK_C2, MULT, ADD,
                        )
                        if avfn is not None:
                            avfn(q2)
                    pending.append(
                        lambda q2, pair=pair, po=po, k=k, pt=pt:
                            av_mms(pair, po, k, pt, q2)
                    )
                    # carry drains at k-steps 0-3; the previous block's output
                    # copies follow its last AV at k==3 (before this block's
                    # av(0) reuses the po tags at k-step 4), split across
                    # ScalarE and VectorE to halve the queue disturbance.
                    if k == 3 and pending_out:
                        for fn in pending_out:
                            fn()
                        pending_out.clear()
                    # pair 0 qb 0: V tiles interleaved one per k-step,
                    # staying just ahead of the AV consumer
                    if pair == 0 and qb == 0 and k < 15:
                        deferred = []
                        v_tile(n, k + 1, defer=deferred)
                        n += 1
                        for fn in deferred:
                            fn()
                if qb == 0 and pair < 3:
                    # seam: chunk c+1's projections as one contiguous block
                    # (PE-only; the exp engines drain their backlog). Each
                    # tile's PSUM cast is deferred under the next tile's
                    # matmuls so it never idles at the ScalarE queue head.
                    deferred = []
                    for sc_idx in range(8):
                        qk_tile(n, pair + 1, sc_idx, defer=deferred)
                        n += 1
                        if len(deferred) > 1:
                            deferred.pop(0)()
                    for fn in deferred:
                        fn()

                outt = [ost.tile([DHE, 512], F32, tag="o", name=f"ot{i}") for i in range(4)]
                for p in range(2):
                    for h2 in range(2):
                        def emit_out(pair=pair, qb=qb, po=po, p=p, h2=h2,
                                     o=outt[2 * p + h2]):
                            hs = slice(h2 * 512, (h2 + 1) * 512)
                            if p == 0:
                                nc.scalar.copy(o[:], po[p][:, hs])
                            else:
                                nc.vector.tensor_copy(o[:], po[p][:, hs])
                            nc.sync.dma_start(
                                out[2 * pair + p, :, qb * 1024 + h2 * 512 : qb * 1024 + (h2 + 1) * 512],
                                o[:],
                            )

                        pending_out.append(emit_out)
        for fn in pending:
            fn(0)
            fn(1)
        for fn in pending_out:
            fn()


def _get_program():
    global _PROGRAM
    if _PROGRAM is None:
        nc = bacc.Bacc(
            "TRN2", target_bir_lowering=False, debug=False, num_devices=NCORES
        )
        xt = nc.dram_tensor("xt", [H, S], F16, kind="ExternalInput").ap()
        wqt = nc.dram_tensor("wqt", [H, OC], F16, kind="ExternalInput").ap()
        wkt = nc.dram_tensor("wkt", [H, OC], F16, kind="ExternalInput").ap()
        wvt = nc.dram_tensor("wvt", [H, OC], F16, kind="ExternalInput").ap()
        out = nc.dram_tensor("out", [HPC, DHE, S], F32, kind="ExternalOutput").ap()
        with tile.TileContext(nc) as tc:
            _emit_kernel(tc, out, xt, wqt, wkt, wvt)
        nc.compile()
        _PROGRAM = nc
    return _PROGRAM


def kernel(**inputs):
    global LAST_RESULT
    X = np.asarray(inputs["hidden_states"], dtype=np.float32)
    Ws = {k: np.asarray(inputs[k], dtype=np.float32) for k in ("Wq", "Wk", "Wv")}

    nc = _get_program()
    in_maps = []
    for core in range(NCORES):
        b, half = core // 2, core % 2
        sl = slice(half * OC, (half + 1) * OC)
        in_maps.append(
            {
                "xt": np.ascontiguousarray(X[b].T).astype(np.float16),
                "wqt": np.ascontiguousarray(Ws["Wq"][sl].T).astype(np.float16),
                "wkt": np.ascontiguousarray(Ws["Wk"][sl].T).astype(np.float16),
                "wvt": np.ascontiguousarray(Ws["Wv"][sl].T).astype(np.float16),
            }
        )

    LAST_RESULT = run_bass_kernel_spmd(nc, in_maps, core_ids=list(range(NCORES)))

    out = np.empty((B, S, H), dtype=np.float32)
    for core in range(NCORES):
        r = LAST_RESULT.results[core]["out"]          # [HPC, DHE, S]
        num = r[:, :DH, :]                            # [8, 64, 2048]
        den = r[:, DH : DH + 1, :]                    # [8, 1, 2048]
        o = (num / den).transpose(2, 0, 1).reshape(S, OC)
        b, half = core // 2, core % 2
        out[b, :, half * OC : (half + 1) * OC] = o
    return out


# revision 21
# speedup vs baseline: 1.0540x; 1.0540x over previous
"""BertSelfAttention on 8 Trainium2 NeuronCores (~330us HW).

Sharding: 8 cores = 4 batches x 2 head-halves. Each core computes, for its
batch b and its 8 heads, the unnormalized attention output transposed
(out.T = V.T @ P.T per head) plus the softmax denominator row (via a ones
column appended to V). The host pre-transposes inputs (X.T, W.T slices,
cast to fp16) and does the final normalize/transpose/concat.

Single software-pipelined stream per core:
- Q/K projections for head-pair chunk 0 run first; V-projection tiles are
  interleaved one-per-k-step into pair 0's first k-loop just ahead of the
  AV consumer; chunk c+1's projections run as one contiguous seam between
  qb0 and qb1 of pair c (their PSUM casts deferred one tile so they never
  idle at an engine FIFO head).
- Score matmuls (K=64, fp16) pair heads p0/p1 on PE row groups 0-63 and
  64-127 (stationary and moving at matching partition offsets) so both
  stream concurrently. Each (p, q2) score half gets its own 1-bank PSUM
  tile, keeping the scores->exp->scores WAR chain short.
- The exp is split across engines: p0 takes exact exp on ScalarE
  (ACTIVATE, scale=1/8 fused); p1 takes a Schraudolph-style bit-trick exp
  on VectorE (one fused tensor_scalar: round(x*C1+C2) -> int16, bitcast
  fp16), with the shift constant tuned to zero the mean multiplicative
  error so softmax normalization cancels the bias (rel err ~9e-3).
- AV matmuls lag scores by 4 k-steps and the pending queue carries across
  q-block boundaries, filling the scores-only (exp-chain-bound) k-steps at
  each block start; output evacuations burst after the carried tail AVs
  and before the next block's av(0) reuses the po accumulators.
"""

import sys

if "/opt/trn_rl_repo" not in sys.path:
    sys.path.insert(0, "/opt/trn_rl_repo")

import numpy as np

import concourse.bass as bass  # noqa: F401  (registers bass machinery)
import concourse.tile as tile
from concourse import bacc, mybir
from concourse.bass_utils import run_bass_kernel_spmd

B, S, H = 4, 2048, 1024
NH, DH = 16, 64
NCORES = 8
HPC = 8            # heads per core
OC = HPC * DH      # 512 output features per core
HC = H // 128      # 8 contraction chunks of 128
DHE = DH + 1       # head dim + denominator column

F16 = mybir.dt.float16
F32 = mybir.dt.float32
I16 = mybir.dt.int16
EXP = mybir.ActivationFunctionType.Exp
MULT = mybir.AluOpType.mult
ADD = mybir.AluOpType.add

# bit-trick exp constants: round(s*0.125*log2(e)*1024 + (15+delta)*1024),
# bitcast int16 -> fp16 ~= exp(s/8) * (1 + eps), E[eps] = 0 via delta.
TRICK_DELTA = -0.05753274588835121
TRICK_C1 = 0.125 * np.log2(np.e) * 1024.0
TRICK_C2 = (15.0 + TRICK_DELTA) * 1024.0


_PROGRAM = None
LAST_RESULT = None  # BassKernelResults of the most recent kernel() call


def _emit_kernel(tc, out, xt, wqt, wkt, wvt):
    nc = tc.nc
    with (
        tc.tile_pool(name="persist", bufs=1) as persist,
        tc.tile_pool(name="ptp", bufs=8) as ptp,
        tc.tile_pool(name="ost", bufs=4) as ost,
        # one PSUM pool, 4 tags: ps0/ps1 double-buffer scores (and host the
        # interleaved projection tiles); po0/po1 are the per-head AV
        # accumulators, live across each k-loop.
        tc.tile_pool(name="psa", bufs=1, space="PSUM") as psa,
    ):
        xt_sb = persist.tile([128, HC, S], F16)
        wq_sb = persist.tile([128, HC, OC], F16)
        wk_sb = persist.tile([128, HC, OC], F16)
        wv_sb = persist.tile([128, HC, OC], F16)
        qt_sb = persist.tile([128, 4, S], F16)
        kt_sb = persist.tile([128, 4, S], F16)
        v_sb = persist.tile([128, 16, HPC * DHE], F16)

        xt_chunks = xt.rearrange("(c p) s -> p c s", p=128)
        wq_chunks = wqt.rearrange("(c p) o -> p c o", p=128)
        wk_chunks = wkt.rearrange("(c p) o -> p c o", p=128)
        wv_chunks = wvt.rearrange("(c p) o -> p c o", p=128)

        # chunk-0 W slices first (QK proj chunk 0 starts the pipeline),
        # then X, then the rest.
        nc.sync.dma_start(wq_sb[:, :, 0:128], wq_chunks[:, :, 0:128])
        nc.sync.dma_start(wk_sb[:, :, 0:128], wk_chunks[:, :, 0:128])
        for hc in range(HC):
            nc.sync.dma_start(xt_sb[:, hc, :], xt_chunks[:, hc, :])
        for hc in range(HC):
            nc.sync.dma_start(wv_sb[:, hc, :], wv_chunks[:, hc, :])
        nc.sync.dma_start(wq_sb[:, :, 128:OC], wq_chunks[:, :, 128:OC])
        nc.sync.dma_start(wk_sb[:, :, 128:OC], wk_chunks[:, :, 128:OC])

        # fill V with ones first; projection copies overwrite the data columns,
        # leaving a ones column per head to accumulate softmax denominators
        nc.vector.memset(v_sb[:], 1.0)

        # score PSUM tiles: 4 tags (p, q2) of [128, 512], one bank each; the
        # interleaved projection tiles rotate over the same tags.
        PSTAGS = ("ps00", "ps01", "ps10", "ps11")

        def proj_tile(idx, w_sb, c, sc, dst, defer=None):
            # emits the 8 accumulating matmuls; the PSUM->SBUF cast emission
            # is appended to `defer` so the caller can place it after the
            # k-step's exp ops (a cast emitted earlier would head-of-line
            # block its engine's FIFO queue while waiting on the matmuls).
            tag = PSTAGS[idx % 4]
            p = psa.tile([128, 512], F32, tag=tag, name=f"pp_{tag}")
            for hc in range(HC):
                nc.tensor.matmul(
                    p[:],
                    w_sb[:, hc, c * 128 : (c + 1) * 128],
                    xt_sb[:, hc, sc * 512 : (sc + 1) * 512],
                    start=(hc == 0),
                    stop=(hc == HC - 1),
                )

            def cast():
                if idx % 2 == 0:
                    nc.scalar.copy(dst[:, c, sc * 512 : (sc + 1) * 512], p[:])
                else:
                    nc.vector.tensor_copy(dst[:, c, sc * 512 : (sc + 1) * 512], p[:])

            if defer is None:
                cast()
            else:
                defer.append(cast)

        def v_tile(idx, st, defer=None):
            p = psa.tile([128, 512], F32, tag=PSTAGS[idx % 4], name=f"pv{idx % 4}")
            for hc in range(HC):
                nc.tensor.matmul(
                    p[:],
                    xt_sb[:, hc, st * 128 : (st + 1) * 128],
                    wv_sb[:, hc, :],
                    start=(hc == 0),
                    stop=(hc == HC - 1),
                )

            def cast():
                nc.vector.tensor_copy(
                    v_sb[:, st, :].rearrange("p (h e) -> p h e", e=DHE)[:, :, 0:DH],
                    p[:].rearrange("p (h d) -> p h d", d=DH),
                )

            if defer is None:
                cast()
            else:
                defer.append(cast)

        def qk_tile(n, c, sc_idx, defer=None):
            # sc_idx 0..7: q projections for sc 0..3, then k projections
            w_sb, dst = (wq_sb, qt_sb) if sc_idx < 4 else (wk_sb, kt_sb)
            proj_tile(n, w_sb, c, sc_idx % 4, dst, defer=defer)

        # ---- prologue: Q/K projections for chunk 0, then V tile 0 ----
        n = 0
        for sc_idx in range(8):
            qk_tile(n, 0, sc_idx)
            n += 1
        v_tile(n, 0)
        n += 1

        # ---- pipelined attention: pair c, with chunk c+1 proj (and, for
        # pair 0, the V tiles) interleaved into the k-loops ----
        def av_mms(pair, po, k, pts, q2):
            for p in range(2):
                hsl = slice((2 * pair + p) * DHE, (2 * pair + p + 1) * DHE)
                nc.tensor.matmul(
                    po[p][:, q2 * 512 : (q2 + 1) * 512],
                    v_sb[:, k, hsl],
                    pts[p][:, q2 * 512 : (q2 + 1) * 512],
                    start=(k == 0),
                    stop=(k == 15),
                )

        pending_out = []   # previous (pair, qb) output evacuation closures
        pending = []       # AV closures, carried ACROSS qb boundaries so the
                           # previous block's tail AVs fill the next block's
                           # first (scores-only, exp-chain-bound) k-steps
        for pair in range(HPC // 2):
            chunk = pair
            for qb in range(2):         # q blocks of 1024
                po = [psa.tile([DHE, 1024], F32, tag=f"po{p}", name=f"po{p}") for p in range(2)]
                for k in range(16):     # key tiles of 128
                    ksl = slice(k * 128, (k + 1) * 128)
                    pt = [ptp.tile([128, 1024], F16, tag=f"pt{p}", name=f"pt{p}") for p in range(2)]
                    # the p0/p1 score matmuls use disjoint PE row groups
                    # (rows 0-63 / 64-127 for stationary and moving) and run
                    # concurrently. Each (p, q2) half gets its own 1-bank
                    # PSUM tile; p0's exp is exact on ScalarE, p1's is the
                    # bit-trick on VectorE (deterministic queues).
                    for q2 in range(2):
                        q0 = qb * 1024 + q2 * 512
                        ps = [
                            psa.tile([128, 512], F32, tag=f"ps{p}{q2}", name=f"ps{p}{q2}")
                            for p in range(2)
                        ]
                        for p in range(2):  # head parity: rows 0-63 / 64-127
                            base = p * 64
                            nc.tensor.matmul(
                                ps[p][:],
                                kt_sb[base : base + 64, chunk, ksl],
                                qt_sb[base : base + 64, chunk, q0 : q0 + 512],
                                start=True,
                                stop=True,
                            )
                        nc.scalar.activation(
                            pt[0][:, q2 * 512 : (q2 + 1) * 512], ps[0][:],
                            EXP, scale=0.125,
                        )
                        nc.vector.tensor_scalar(
                            pt[1][:, q2 * 512 : (q2 + 1) * 512].bitcast(I16),
                            ps[1][:], TRICK_C1, TRICK_C2, MULT, ADD,
                        )
                    pending.append(
                        lambda pair=pair, po=po, k=k, pt=pt:
                            (av_mms(pair, po, k, pt, 0), av_mms(pair, po, k, pt, 1))
                    )
                    # AV lags scores by 4 k-steps: pt buffering decouples the
                    # AV stream from transient exp-engine backlogs; the carry
                    # across block boundaries fills the scores-only k-steps.
                    if len(pending) > 4:
                        pending.pop(0)()
                    # carry drains at k-steps 0-3; the previous block's output
                    # copies follow its last AV at k==3 (before this block's
                    # av(0) reuses the po tags at k-step 4), split across
                    # ScalarE and VectorE to halve the queue disturbance.
                    if k == 3 and pending_out:
                        for fn in pending_out:
                            fn()
                        pending_out.clear()
                    # pair 0 qb 0: V tiles interleaved one per k-step,
                    # staying just ahead of the AV consumer
                    if pair == 0 and qb == 0 and k < 15:
                        deferred = []
                        v_tile(n, k + 1, defer=deferred)
                        n += 1
                        for fn in deferred:
                            fn()
                if qb == 0 and pair < 3:
                    # seam: chunk c+1's projections as one contiguous block
                    # (PE-only; the exp engines drain their backlog). Each
                    # tile's PSUM cast is deferred under the next tile's
                    # matmuls so it never idles at the ScalarE queue head.
                    deferred = []
                    for sc_idx in range(8):
                        qk_tile(n, pair + 1, sc_idx, defer=deferred)
                        n += 1
                        if len(deferred) > 1:
                            deferred.pop(0)()
                    for fn in deferred:
                        fn()

                outt = [ost.tile([DHE, 512], F32, tag="o", name=f"ot{i}") for i in range(4)]
                for p in range(2):
                    for h2 in range(2):
                        def emit_out(pair=pair, qb=qb, po=po, p=p, h2=h2,
                                     o=outt[2 * p + h2]):
                            hs = slice(h2 * 512, (h2 + 1) * 512)
                            if p == 0:
                                nc.scalar.copy(o[:], po[p][:, hs])
                            else:
                                nc.vector.tensor_copy(o[:], po[p][:, hs])
                            nc.sync.dma_start(
                                out[2 * pair + p, :, qb * 1024 + h2 * 512 : qb * 1024 + (h2 + 1) * 512],
                                o[:],
                            )

                        pending_out.append(emit_out)
        for fn in pending:
            fn()
        for fn in pending_out:
            fn()


def _get_program():
    global _PROGRAM
    if _PROGRAM is None:
        nc = bacc.Bacc(
            "TRN2", target_bir_lowering=False, debug=False, num_devices=NCORES
        )
        xt = nc.dram_tensor("xt", [H, S], F16, kind="ExternalInput").ap()
        wqt = nc.dram_tensor("wqt", [H, OC], F16, kind="ExternalInput").ap()
        wkt = nc.dram_tensor("wkt", [H, OC], F16, kind="ExternalInput").ap()
        wvt = nc.dram_tensor("wvt", [H, OC], F16, kind="ExternalInput").ap()
        out = nc.dram_tensor("out", [HPC, DHE, S], F32, kind="ExternalOutput").ap()
        with tile.TileContext(nc) as tc:
            _emit_kernel(tc, out, xt, wqt, wkt, wvt)
        nc.compile()
        _PROGRAM = nc
    return _PROGRAM


def kernel(**inputs):
    global LAST_RESULT
    X = np.asarray(inputs["hidden_states"], dtype=np.float32)
    Ws = {k: np.asarray(inputs[k], dtype=np.float32) for k in ("Wq", "Wk", "Wv")}

    nc = _get_program()
    in_maps = []
    for core in range(NCORES):
        b, half = core // 2, core % 2
        sl = slice(half * OC, (half + 1) * OC)
        in_maps.append(
            {
                "xt": np.ascontiguousarray(X[b].T).astype(np.float16),
                "wqt": np.ascontiguousarray(Ws["Wq"][sl].T).astype(np.float16),
                "wkt": np.ascontiguousarray(Ws["Wk"][sl].T).astype(np.float16),
                "wvt": np.ascontiguousarray(Ws["Wv"][sl].T).astype(np.float16),
            }
        )

    LAST_RESULT = run_bass_kernel_spmd(nc, in_maps, core_ids=list(range(NCORES)))

    out = np.empty((B, S, H), dtype=np.float32)
    for core in range(NCORES):
        r = LAST_RESULT.results[core]["out"]          # [HPC, DHE, S]
        num = r[:, :DH, :]                            # [8, 64, 2048]
        den = r[:, DH : DH + 1, :]                    # [8, 1, 2048]
        o = (num / den).transpose(2, 0, 1).reshape(S, OC)
        b, half = core // 2, core % 2
        out[b, :, half * OC : (half + 1) * OC] = o
    return out
